# revision 17
# baseline (speedup 1.0000x reference)
"""2D Haar DWT (mode=0 'even') on Trainium2, 8 NeuronCores.

Input : x [2, 16, 16, 256, 256] f32, mode (0)
Output: [2, 64, 16, 128, 128] f32  (channel concat of LL, HL, LH, HH)

x is cast to bf16 on the host before upload (input HBM bytes halve:
33.55 -> 25.17 MB/core total stream, taking even HBM-contended cores
below the DVE compute bound).  The 0.5 prescale is exact in bf16
(exponent shift); all arithmetic runs fp32 internally on DVE; output
stays f32.  l2 rel err 1.7e-3 (gate 2e-2).

Sharding: the 2*16 = 32 (b, c) pairs are split 4-per-core across 8 cores.
Each core processes 4 groups x 16 depth-images of 256x256 and emits, for
each group, four subband stacks [16, 128, 128] that are contiguous slices
of the full output (y[b, s*16+c, :, :, :]). No inter-core communication.

Per-core kernel (Tile framework), 8 iterations of 8 depth-images each:
  - partition p = (j, q): image j in [0,8) x 16-row block q in [0,16)
    so each partition holds 16 consecutive input rows (16 KiB contiguous
    DRAM per partition per input DMA) and produces 8 consecutive output
    rows (4 KiB contiguous DRAM per partition+subband on the write).
  - Sync issues input DMAs (HWDGE Q1); Scalar/ACT halves the ODD rows in
    place and issues ONE merged output DMA per chunk (HWDGE Q10, all four
    subbands from one out tile), emitted with a 1-chunk lag so prescales
    never queue behind DVE waits.
  - DVE does all butterfly math (GpSimd/PE offload was tried and hurts:
    concurrent GpSimd tensor ops degrade DVE SBUF throughput ~1.5x).
    Stage 1 via scalar_tensor_tensor folds the even rows' 0.5:
      vs = (a * 0.5) + b'      vd = (a * -0.5) + b'      (b' = b/2)
    Stage 2 is merged across the vs/vd halves of one mid tile (the
    (e, w2) dims fuse into one uniform stride-2 dim, so ops stay at the
    proven 1 elem/lane/cycle f32 rate):
      [LL|LH] = even cols + odd cols    [HL|HH] = odd cols - even cols
"""

import numpy as np

N_CORES = 8
B, C, D, H, W = 2, 16, 16, 256, 256
GROUPS_PER_CORE = 4  # (b,c) pairs per core
D_SPLIT = 2          # halves of the depth dim per group
D_SUB = D // D_SPLIT # images per iteration (8)

_compiled_nc = None


def _build_nc():
    import concourse.bacc as bacc
    import concourse.tile as tile
    import concourse.mybir as mybir

    f32 = mybir.dt.float32
    bf16 = mybir.dt.bfloat16
    nc = bacc.Bacc("TRN2", target_bir_lowering=False, debug=False,
                   num_devices=N_CORES)

    x = nc.dram_tensor("x", [GROUPS_PER_CORE, D, H, W], bf16,
                       kind="ExternalInput")
    y = nc.dram_tensor("y", [GROUPS_PER_CORE, 4, D, H // 2, W // 2], f32,
                       kind="ExternalOutput")

    # partition p = (j, q): image j (8), 16-row block q (16)
    # [8 iter, 128 part, 16 row, 256 w]; 16 KiB contiguous per partition
    xa = x.rearrange("g (i j) (q sixteen) w -> (g i) (j q) sixteen w",
                     i=D_SPLIT, j=D_SUB, q=16, sixteen=16)
    # output rows h = 8q + e; 4 KiB contiguous per partition+subband
    # [4 grp, 2 half, 128 part, 4 subband, 8 e, 128 w]
    ya = y.rearrange("bc s (i j) (q e) w -> bc i (j q) s e w",
                     i=D_SPLIT, j=D_SUB, q=16, e=8)

    n_iters = GROUPS_PER_CORE * D_SPLIT
    W2 = W // 2  # 128

    alu = mybir.AluOpType

    with tile.TileContext(nc) as tc:
        with tc.tile_pool(name="io", bufs=5) as io_pool, \
             tc.tile_pool(name="mid", bufs=3) as mid_pool, \
             tc.tile_pool(name="outp", bufs=4) as out_pool:
            # output DMAs (on ACT's HWDGE ring) are emitted with a
            # 1-chunk lag so ACT prescales never queue behind DVE waits
            pending = []  # (yc, r0, r1, o_tile, ne)

            def flush(keep):
                while len(pending) > keep:
                    yc_, r0_, r1_, ot_, ne_ = pending.pop(0)
                    ov = ot_[:].rearrange("p (s e w2) -> p s e w2",
                                          s=4, e=ne_)
                    nc.scalar.dma_start(
                        yc_[:, :, r0_ // 2:r1_ // 2, :], ov)

            for it in range(n_iters):
                # first iteration split to start compute sooner; last
                # iteration in smaller row-chunks to shrink the exposed
                # compute tail after the final input lands
                if it == 0 or it == n_iters - 2:
                    chunks = [(0, 8), (8, 16)]
                elif it == n_iters - 1:
                    chunks = [(0, 8), (8, 12), (12, 16)]
                else:
                    chunks = [(0, 16)]
                for r0, r1 in chunks:
                    nr = r1 - r0
                    ne = nr // 2
                    half = ne * W2
                    t_in = io_pool.tile([128, nr * W], bf16, tag="t_in")
                    t_in_v = t_in[:].rearrange("p (r w) -> p r w", r=nr)
                    nc.sync.dma_start(t_in_v, xa[it, :, r0:r1, :])

                    # rows r = 2e + par; even/odd row views [128, ne, 256]
                    tv = t_in[:].rearrange("p (e par w) -> p e par w",
                                           e=ne, par=2)
                    a_v = tv[:, :, 0, :]
                    b_v = tv[:, :, 1, :]
                    # 0.5 prescale on ACT for the ODD rows only; the even
                    # rows' 0.5 is folded into the DVE stage-1 STT ops
                    nc.scalar.mul(b_v, b_v, 0.5)

                    # row butterfly into one mid tile m = [vs | vd]
                    # vs = a/2 + b', vd = b' - a/2  (b' = b/2)
                    m = mid_pool.tile([128, 2 * ne * W], f32, tag="m")
                    mv = m[:].rearrange("p (b e w) -> p b e w", b=2, e=ne)
                    nc.vector.scalar_tensor_tensor(
                        mv[:, 0], a_v, 0.5, b_v, alu.mult, alu.add)
                    nc.vector.scalar_tensor_tensor(
                        mv[:, 1], a_v, -0.5, b_v, alu.mult, alu.add)

                    # col butterfly, merged across the vs/vd branches:
                    # cols w = 2*x + cp, x = (e, w2) merged (uniform
                    # stride 2); in views [p, b, x]
                    mc = m[:].rearrange("p (b x cp) -> p b x cp",
                                        b=2, cp=2)
                    # one out tile per chunk, subband-ordered
                    # [LL | HL | LH | HH] = y s 0..3, ONE write DMA
                    o = out_pool.tile([128, 4 * half], f32, tag="o")
                    o4 = o[:].rearrange("p (s x) -> p s x", s=4)
                    # adds: LL (from vs) -> s0, LH (from vd) -> s2
                    nc.vector.tensor_add(o4[:, 0::2, :],
                                         mc[:, :, :, 0], mc[:, :, :, 1])
                    # subs: HL (from vs) -> s1, HH (from vd) -> s3
                    nc.vector.tensor_sub(o4[:, 1::2, :],
                                         mc[:, :, :, 1], mc[:, :, :, 0])
                    yc = ya[it // D_SPLIT, it % D_SPLIT]
                    pending.append((yc, r0, r1, o, ne))
                    flush(keep=1)
            flush(keep=0)

    nc.compile()
    return nc


def _get_nc():
    global _compiled_nc
    if _compiled_nc is None:
        _compiled_nc = _build_nc()
    return _compiled_nc


def _haar_numpy(x):
    # mode='odd' fallback: pad one zero row/col at the end of H and W
    x = np.pad(x, ((0, 0), (0, 0), (0, 0), (0, 1), (0, 1)))
    x01 = x[:, :, :, 0::2, :] * 0.5
    x02 = x[:, :, :, 1::2, :] * 0.5
    x1 = x01[..., 0::2]
    x2 = x02[..., 0::2]
    x3 = x01[..., 1::2]
    x4 = x02[..., 1::2]
    return np.concatenate((x1 + x2 + x3 + x4, -x1 - x2 + x3 + x4,
                           -x1 + x2 - x3 + x4, x1 - x2 - x3 + x4), axis=1)


def run_device(in_maps, trace=False, **kwargs):
    """Run the compiled SPMD kernel; returns BassKernelResults."""
    from concourse.bass_utils import run_bass_kernel_spmd
    nc = _get_nc()
    return run_bass_kernel_spmd(nc, in_maps, core_ids=list(range(N_CORES)),
                                trace=trace, **kwargs)


_cached_exec = None  # (callable, out_shape) reused across kernel() calls


def _get_cached_exec():
    """Build the sharded PJRT executable once; jax caches its compilation
    across calls (run_bass_via_pjrt rebuilds the jit closure every call,
    paying retrace + XLA lowering each time)."""
    global _cached_exec
    if _cached_exec is not None:
        return _cached_exec
    import jax
    from jax.experimental.shard_map import shard_map
    from jax.sharding import Mesh, PartitionSpec
    from concourse import bass2jax

    bass2jax.install_neuronx_cc_hook()
    nc = _get_nc()
    out_shape = (GROUPS_PER_CORE, 4, D, H // 2, W // 2)
    out_aval = jax.core.ShapedArray(out_shape, np.float32)

    def _body(x_arg, y_zero):
        outs = bass2jax._bass_exec_p.bind(
            x_arg, y_zero,
            out_avals=(out_aval,),
            in_names=("x", "y"),
            out_names=("y",),
            lowering_input_output_aliases=(),
            sim_require_finite=True,
            sim_require_nnan=True,
            nc=nc,
        )
        return (outs[0],)

    devices = jax.devices()[:N_CORES]
    mesh = Mesh(np.asarray(devices), ("core",))
    fn = jax.jit(
        shard_map(_body, mesh=mesh,
                  in_specs=(PartitionSpec("core"),) * 2,
                  out_specs=(PartitionSpec("core"),),
                  check_rep=False),
        donate_argnums=(1,), keep_unused=True)
    _cached_exec = (fn, out_shape)
    return _cached_exec


def make_in_maps(x):
    import ml_dtypes
    xs = np.ascontiguousarray(np.asarray(x, dtype=np.float32)
                              .astype(ml_dtypes.bfloat16)
                              .reshape(B * C, D, H, W))
    return [{"x": xs[GROUPS_PER_CORE * k: GROUPS_PER_CORE * (k + 1)]}
            for k in range(N_CORES)]


def gather_output(results):
    out = np.stack([results[k]["y"] for k in range(N_CORES)])
    # [8, 4, 4, 16, 128, 128] -> [b, c, s, d, h, w] -> [b, s*16+c, d, h, w]
    out = out.reshape(B, C, 4, D, H // 2, W // 2)
    out = out.transpose(0, 2, 1, 3, 4, 5).reshape(B, 4 * C, D,
                                                  H // 2, W // 2)
    return np.ascontiguousarray(out)


def _run_fast(x):
    import ml_dtypes
    fn, out_shape = _get_cached_exec()
    xs = np.ascontiguousarray(np.asarray(x, dtype=np.float32)
                              .astype(ml_dtypes.bfloat16)
                              .reshape(B * C, D, H, W))
    zeros = np.zeros((N_CORES * out_shape[0], *out_shape[1:]), np.float32)
    (y,) = fn(xs, zeros)
    out = np.asarray(y).reshape(B, C, 4, D, H // 2, W // 2)
    out = out.transpose(0, 2, 1, 3, 4, 5).reshape(B, 4 * C, D,
                                                  H // 2, W // 2)
    return np.ascontiguousarray(out)


def kernel(x, mode):
    mode_val = int(np.asarray(mode))
    if mode_val != 0:
        return _haar_numpy(np.asarray(x, dtype=np.float32))
    try:
        return _run_fast(x)
    except Exception:
        pass  # fall back to the stock bass_utils path below
    in_maps = make_in_maps(x)
    try:
        res = run_device(in_maps)
    except Exception:
        res = run_device(in_maps)  # one retry for transient device errors
    return gather_output(res.results)
